# revision 1
# baseline (speedup 1.0000x reference)
"""GNN message passing (copy_u + segment_sum) on 8 Trainium2 cores.

Strategy (edge/data parallel, per the sharding hint):
  - Host: sort edges by dst; core c owns dst range [c*N/8, (c+1)*N/8).
  - Host: pack src_emb rows as [bf16(hi) | bf16(src-hi)] pairs (256B/row, exact
    to ~1e-5 rel) and gather per-edge message rows per core ("src_emb ...
    gathered per partition"), laid out partition-major so device DMAs are
    contiguous 32KB runs per partition.
  - Device (per core): stream message tiles; for each bin of <=128 dst rows /
    S*128 edge slots, build a one-hot [edge x dst-slot] matrix on DVE
    (dstloc == iota), then segment-sum via PE matmuls accumulating hi+lo into
    one PSUM bank; copy PSUM->SBUF, batch-store bins per group.
  - Host: scatter-add the [B*128, 64] bin blocks back to the full output.
"""
import sys
sys.path.insert(0, "/opt/trn_rl_repo")
import numpy as np
import ml_dtypes

import concourse.bass as bass
import concourse.bacc as bacc
import concourse.mybir as mybir
import concourse.tile as tile
from concourse.bass_utils import run_bass_kernel_spmd

NCORES = 8
S = 9                # subtiles (of 128 edge slots) per bin
CAP = S * 128        # edge slots per bin
PAD_LOC = 200.0      # dstloc sentinel -> one-hot row all zeros
BF16 = ml_dtypes.bfloat16

_kernel_cache = {}


def _build_kernel(B):
    """Device program: uniform over cores; B bins of S subtiles each."""
    bf16 = mybir.dt.bfloat16
    f32 = mybir.dt.float32
    nc = bacc.Bacc("TRN2", target_bir_lowering=False, debug=False,
                   num_devices=NCORES)
    msg = nc.declare_dram_parameter("msg", [128, B * CAP], bf16, isOutput=False)
    dstloc = nc.declare_dram_parameter("dstloc", [128, B * S], bf16, isOutput=False)
    iota = nc.declare_dram_parameter("iota", [128, CAP], bf16, isOutput=False)
    outp = nc.declare_dram_parameter("outp", [128, B * 64], f32, isOutput=True)

    G = 14  # bins per DMA group
    n_groups = (B + G - 1) // G

    with tile.TileContext(nc) as tc:
        with tc.tile_pool(name="const", bufs=1) as cpool, \
             tc.tile_pool(name="msgs", bufs=3) as mpool, \
             tc.tile_pool(name="oh", bufs=4) as ohpool, \
             tc.tile_pool(name="acc", bufs=8, space="PSUM") as ppool, \
             tc.tile_pool(name="ost", bufs=3) as opool:
            iota_t = cpool.tile([128, CAP], bf16)
            nc.sync.dma_start(out=iota_t[:], in_=iota[:])
            dstloc_t = cpool.tile([128, B * S], bf16)
            nc.sync.dma_start(out=dstloc_t[:], in_=dstloc[:])
            iota3d = iota_t[:].rearrange("p (s f) -> p s f", s=S)

            for g in range(n_groups):
                g0 = g * G
                gs = min(G, B - g0)
                mt = mpool.tile([128, gs * CAP], bf16, tag="mt")
                nc.sync.dma_start(out=mt[:], in_=msg[:, g0 * CAP:(g0 + gs) * CAP])
                ot = opool.tile([128, gs * 64], f32, tag="ot")
                for lb in range(gs):
                    b = g0 + lb
                    oh = ohpool.tile([128, S, 128], bf16)
                    nc.vector.tensor_tensor(
                        out=oh[:],
                        in0=dstloc_t[:, b * S:(b + 1) * S].to_broadcast([128, S, 128]),
                        in1=iota3d,
                        op=mybir.AluOpType.is_equal,
                    )
                    ps = ppool.tile([128, 64], f32)
                    for s in range(S):
                        base = lb * CAP + s * 128
                        nc.tensor.matmul(ps[:], oh[:, s, :], mt[:, base:base + 64],
                                         start=(s == 0), stop=False)
                        nc.tensor.matmul(ps[:], oh[:, s, :], mt[:, base + 64:base + 128],
                                         start=False, stop=(s == S - 1))
                    nc.vector.tensor_copy(out=ot[:, lb * 64:(lb + 1) * 64], in_=ps[:])
                nc.sync.dma_start(out=outp[:, g0 * 64:(g0 + gs) * 64], in_=ot[:])
    nc.compile()
    return nc


def _pack_core(d_local, s_local, n_dst_local):
    """Greedy bins: <=128 distinct dst rows and <=CAP edges per bin.
    Returns (srcs [B,CAP] int64, locs [B,CAP] uint8->float, rows [B,128] int64
    with n_dst_local as trash)."""
    n = len(d_local)
    bins = []
    if n:
        firsts = np.flatnonzero(np.concatenate(([True], d_local[1:] != d_local[:-1])))
        nf = len(firsts)
        start = 0
        while start < n:
            j0 = np.searchsorted(firsts, start, side="right") - 1
            lim = firsts[j0 + 128] if j0 + 128 < nf else n
            end = min(start + CAP, lim)
            bins.append((start, end))
            start = end
    B = len(bins)
    srcs = np.zeros((B, CAP), dtype=np.int64)
    locs = np.full((B, CAP), PAD_LOC, dtype=np.float32)
    rows = np.full((B, 128), n_dst_local, dtype=np.int64)
    for i, (st, en) in enumerate(bins):
        m = en - st
        u, inv = np.unique(d_local[st:en], return_inverse=True)
        srcs[i, :m] = s_local[st:en]
        locs[i, :m] = inv.astype(np.float32)
        rows[i, :len(u)] = u
    return srcs, locs, rows


def kernel(src_emb, edge_src, edge_dst, num_dst):
    src_emb = np.asarray(src_emb, dtype=np.float32)
    edge_src = np.asarray(edge_src).astype(np.int64)
    edge_dst = np.asarray(edge_dst).astype(np.int64)
    n_dst = int(num_dst)
    n_src, d = src_emb.shape
    assert d == 64

    # hi/lo bf16 split: hi + lo == src exactly to ~2^-17 relative
    hi = src_emb.astype(BF16)
    lo = (src_emb - hi.astype(np.float32)).astype(BF16)
    packed = np.concatenate([hi, lo], axis=1)  # [n_src, 128] bf16

    # dst-sorted edge partition across cores
    order = np.argsort(edge_dst, kind="stable")
    ds = edge_dst[order]
    ss = edge_src[order]
    per = (n_dst + NCORES - 1) // NCORES
    cuts = np.searchsorted(ds, np.arange(1, NCORES) * per)
    d_parts = np.split(ds, cuts)
    s_parts = np.split(ss, cuts)

    cores = []
    for c in range(NCORES):
        dl = d_parts[c] - c * per
        nl = min(per, n_dst - c * per)
        cores.append(_pack_core(dl, s_parts[c], nl))
    B = max(cr[0].shape[0] for cr in cores)

    iota_np = np.tile(np.arange(128, dtype=np.float32), S)[None, :].repeat(128, 0).astype(BF16)

    in_maps = []
    rows_g = []
    for c, (srcs, locs, rows) in enumerate(cores):
        b0 = srcs.shape[0]
        if b0 < B:
            srcs = np.concatenate([srcs, np.zeros((B - b0, CAP), np.int64)])
            locs = np.concatenate([locs, np.full((B - b0, CAP), PAD_LOC, np.float32)])
            nl = min(per, n_dst - c * per)
            rows = np.concatenate([rows, np.full((B - b0, 128), nl, np.int64)])
        # [128, B*S*128] partition-major messages
        msg_np = packed[srcs.reshape(B * S, 128).T].reshape(128, -1)
        dstloc_np = locs.reshape(B * S, 128).T.astype(BF16).copy()
        in_maps.append({"msg": msg_np, "dstloc": dstloc_np, "iota": iota_np})
        nl = min(per, n_dst - c * per)
        # local trash sentinel nl -> dedicated global trash slot n_dst + c
        rows_g.append(np.where(rows == nl, n_dst + c, rows + c * per))

    if B not in _kernel_cache:
        _kernel_cache[B] = _build_kernel(B)
    nc = _kernel_cache[B]
    res = run_bass_kernel_spmd(nc, in_maps, core_ids=list(range(NCORES)))

    full = np.zeros((n_dst + NCORES, 64), dtype=np.float32)
    for c in range(NCORES):
        blocks = res.results[c]["outp"].reshape(128, B, 64).transpose(1, 0, 2)
        np.add.at(full, rows_g[c].ravel(), blocks.reshape(B * 128, 64))
    return full[:n_dst]


if __name__ == "__main__":
    rng = np.random.default_rng(1)
    ns, nd, e = 1000, 1000, 5000
    semb = rng.standard_normal((ns, 64), dtype=np.float32)
    es = rng.integers(0, ns, e)
    ed = rng.integers(0, nd, e)
    got = kernel(src_emb=semb, edge_src=es, edge_dst=ed, num_dst=nd)
    exp = np.zeros((nd, 64), np.float32)
    np.add.at(exp, ed, semb[es])
    rel = np.abs(got - exp).max() / np.abs(exp).max()
    print("small-case rel err:", rel)



# revision 3
# speedup vs baseline: 1.8410x; 1.8410x over previous
"""GNN message passing (copy_u + segment_sum) on 8 Trainium2 cores.

Strategy (edge/data parallel, per the sharding hint):
  - Host: sort dst nodes by degree (desc); tiles of 128 dst rows each get a
    uniform slab depth L = max degree in tile.  Messages for tile t are packed
    [128 partitions = dst slot, 64 feat x L slabs] bf16 with the slab axis
    innermost and zero padding for short segments.
  - Tiles are dealt round-robin to the 8 cores so every core runs the same
    program (rank j's depth = max L over that rank's 8 tiles).
  - Device: per tile one DVE tensor_reduce (add, axis=X) over [128, 64, L]
    -> [128, 64] f32.  No matmul, no one-hot; DMA-bound streaming.
  - Host: scatter rows back (each dst lives in exactly one tile row).
"""
import sys
sys.path.insert(0, "/opt/trn_rl_repo")
import numpy as np
import ml_dtypes

import concourse.bass as bass
import concourse.bacc as bacc
import concourse.mybir as mybir
import concourse.tile as tile
from concourse.bass_utils import run_bass_kernel_spmd

NCORES = 8
GROUP = 14           # ranks per DMA group
BF16 = ml_dtypes.bfloat16

_kernel_cache = {}


def _build_kernel(L_ranks):
    bf16 = mybir.dt.bfloat16
    f32 = mybir.dt.float32
    T = len(L_ranks)
    cols = 64 * int(sum(L_ranks))
    nc = bacc.Bacc("TRN2", target_bir_lowering=False, debug=False,
                   num_devices=NCORES)
    msg = nc.declare_dram_parameter("msg", [128, cols], bf16, isOutput=False)
    outp = nc.declare_dram_parameter("outp", [128, T * 64], f32, isOutput=True)

    offs = np.concatenate(([0], np.cumsum([64 * L for L in L_ranks])))
    n_groups = (T + GROUP - 1) // GROUP

    with tile.TileContext(nc) as tc:
        with tc.tile_pool(name="msgs", bufs=3) as mpool, \
             tc.tile_pool(name="ost", bufs=3) as opool:
            for g in range(n_groups):
                g0 = g * GROUP
                gs = min(GROUP, T - g0)
                goff = int(offs[g0])
                gcols = int(offs[g0 + gs] - offs[g0])
                mt = mpool.tile([128, gcols], bf16, tag="mt")
                nc.sync.dma_start(out=mt[:], in_=msg[:, goff:goff + gcols])
                ot = opool.tile([128, gs * 64], f32, tag="ot")
                for j in range(gs):
                    L = int(L_ranks[g0 + j])
                    loc = int(offs[g0 + j] - goff)
                    nc.vector.tensor_reduce(
                        out=ot[:, j * 64:(j + 1) * 64],
                        in_=mt[:, loc:loc + 64 * L].rearrange(
                            "p (f l) -> p f l", l=L),
                        axis=mybir.AxisListType.X,
                        op=mybir.AluOpType.add,
                    )
                nc.scalar.dma_start(out=outp[:, g0 * 64:(g0 + gs) * 64],
                                    in_=ot[:])
    nc.compile()
    return nc


def kernel(src_emb, edge_src, edge_dst, num_dst):
    src_emb = np.asarray(src_emb, dtype=np.float32)
    edge_src = np.asarray(edge_src).astype(np.int64)
    edge_dst = np.asarray(edge_dst).astype(np.int64)
    n_dst = int(num_dst)
    n_src, d = src_emb.shape
    assert d == 64
    E = len(edge_dst)

    src_ext = np.concatenate(
        [src_emb.astype(BF16), np.zeros((1, 64), BF16)])  # zero row at n_src

    counts = np.bincount(edge_dst, minlength=n_dst)
    order = np.argsort(edge_dst, kind="stable")
    ss = edge_src[order]                      # edge srcs sorted by dst
    starts = np.zeros(n_dst + 1, dtype=np.int64)
    starts[1:] = np.cumsum(counts)

    sort_dst = np.argsort(-counts, kind="stable")
    sorted_counts = counts[sort_dst]

    nnz = int((counts > 0).sum())
    n_tiles = (nnz + 127) // 128              # tiles with at least one edge
    T_pad = (n_tiles + NCORES - 1) // NCORES  # ranks (tiles per core)

    # pad dst list so every (rank, core) has 128 rows; sentinel row = n_dst
    rows_all = np.full(T_pad * NCORES * 128, n_dst, dtype=np.int64)
    take = min(n_dst, n_tiles * 128)
    rows_all[:take] = sort_dst[:take]
    rows_all = rows_all.reshape(T_pad, NCORES, 128)

    counts_pad = np.concatenate([counts, [0]])
    starts_pad = np.concatenate([starts[:-1], [0]])

    L_ranks = tuple(
        int(max(sorted_counts[min(NCORES * j * 128, n_dst - 1)], 1))
        for j in range(T_pad))
    cols = 64 * int(sum(L_ranks))
    offs = np.concatenate(([0], np.cumsum([64 * L for L in L_ranks])))

    msgs = [np.zeros((128, cols), dtype=BF16) for _ in range(NCORES)]
    ar = np.arange(max(L_ranks))
    for j in range(T_pad):
        L = L_ranks[j]
        rows = rows_all[j].reshape(-1)                     # [8*128]
        st = starts_pad[rows]
        cnt = counts_pad[rows]
        eidx = st[:, None] + ar[None, :L]
        valid = ar[None, :L] < cnt[:, None]
        sidx = np.where(valid, ss[np.minimum(eidx, E - 1)], n_src)
        vals = src_ext[sidx]                               # [1024, L, 64]
        block = vals.reshape(NCORES, 128, L, 64).transpose(0, 1, 3, 2)
        block = block.reshape(NCORES, 128, 64 * L)
        o0, o1 = int(offs[j]), int(offs[j + 1])
        for c in range(NCORES):
            msgs[c][:, o0:o1] = block[c]

    if L_ranks not in _kernel_cache:
        _kernel_cache[L_ranks] = _build_kernel(L_ranks)
    nc = _kernel_cache[L_ranks]
    in_maps = [{"msg": msgs[c]} for c in range(NCORES)]
    res = run_bass_kernel_spmd(nc, in_maps, core_ids=list(range(NCORES)))

    full = np.zeros((n_dst + 1, 64), dtype=np.float32)
    for c in range(NCORES):
        blocks = np.asarray(res.results[c]["outp"]).reshape(128, T_pad, 64)
        blocks = blocks.transpose(1, 0, 2).reshape(-1, 64)  # [T_pad*128, 64]
        full[rows_all[:, c, :].reshape(-1)] = blocks
    return full[:n_dst]


if __name__ == "__main__":
    rng = np.random.default_rng(1)
    ns, nd, e = 1000, 1000, 5000
    semb = rng.standard_normal((ns, 64), dtype=np.float32)
    es = rng.integers(0, ns, e)
    ed = rng.integers(0, nd, e)
    got = kernel(src_emb=semb, edge_src=es, edge_dst=ed, num_dst=nd)
    exp = np.zeros((nd, 64), np.float32)
    np.add.at(exp, ed, semb[es])
    rel = np.abs(got - exp).max() / np.abs(exp).max()
    print("small-case rel err:", rel)


# revision 4
# speedup vs baseline: 1.9559x; 1.0624x over previous
"""GNN message passing (copy_u + segment_sum) on 8 Trainium2 cores.

Strategy (edge/data parallel, per the sharding hint):
  - Host: sort dst nodes by degree (desc); tiles of 128 dst rows each get a
    uniform slab depth L = max degree in tile.  Messages for tile t are packed
    slab-major [128 partitions = dst slot, L slabs x 64 feat] bf16 with zero
    padding for short segments.
  - Tiles are dealt round-robin to the 8 cores so every core runs the same
    program (rank j's depth = max L over that rank's 8 tiles).
  - Device: per tile, segment-sum = binary-tree halving with full-width
    unit-stride DVE tensor_tensor adds in place (eligible for 4x DVE mode);
    final level writes bf16 into the output tile.  No matmul, no one-hot;
    the kernel is DMA-bound streaming.
  - Host: scatter rows back (each dst lives in exactly one tile row).
"""
import sys
sys.path.insert(0, "/opt/trn_rl_repo")
import numpy as np
import ml_dtypes

import concourse.bass as bass
import concourse.bacc as bacc
import concourse.mybir as mybir
import concourse.tile as tile
from concourse.bass_utils import run_bass_kernel_spmd

NCORES = 8
GROUP = 14           # ranks per DMA group
BF16 = ml_dtypes.bfloat16

_kernel_cache = {}


def _build_kernel(L_ranks):
    bf16 = mybir.dt.bfloat16
    T = len(L_ranks)
    cols = 64 * int(sum(L_ranks))
    nc = bacc.Bacc("TRN2", target_bir_lowering=False, debug=False,
                   num_devices=NCORES)
    msg = nc.declare_dram_parameter("msg", [128, cols], bf16, isOutput=False)
    outp = nc.declare_dram_parameter("outp", [128, T * 64], bf16, isOutput=True)

    offs = np.concatenate(([0], np.cumsum([64 * L for L in L_ranks])))
    n_groups = (T + GROUP - 1) // GROUP

    with tile.TileContext(nc) as tc:
        with tc.tile_pool(name="msgs", bufs=3) as mpool, \
             tc.tile_pool(name="ost", bufs=3) as opool:
            for g in range(n_groups):
                g0 = g * GROUP
                gs = min(GROUP, T - g0)
                goff = int(offs[g0])
                gcols = int(offs[g0 + gs] - offs[g0])
                mt = mpool.tile([128, gcols], bf16, tag="mt")
                nc.sync.dma_start(out=mt[:], in_=msg[:, goff:goff + gcols])
                ot = opool.tile([128, gs * 64], bf16, tag="ot")
                for j in range(gs):
                    L = int(L_ranks[g0 + j])
                    loc = int(offs[g0 + j] - goff)
                    osl = ot[:, j * 64:(j + 1) * 64]
                    n = L
                    while n > 2:
                        h = n // 2
                        keep = n - h
                        nc.vector.tensor_tensor(
                            out=mt[:, loc:loc + h * 64],
                            in0=mt[:, loc:loc + h * 64],
                            in1=mt[:, loc + keep * 64:loc + n * 64],
                            op=mybir.AluOpType.add,
                        )
                        n = keep
                    if n == 2:
                        nc.vector.tensor_tensor(
                            out=osl,
                            in0=mt[:, loc:loc + 64],
                            in1=mt[:, loc + 64:loc + 128],
                            op=mybir.AluOpType.add,
                        )
                    else:
                        nc.vector.tensor_copy(out=osl, in_=mt[:, loc:loc + 64])
                nc.scalar.dma_start(out=outp[:, g0 * 64:(g0 + gs) * 64],
                                    in_=ot[:])
    nc.compile()
    return nc


def kernel(src_emb, edge_src, edge_dst, num_dst):
    src_emb = np.asarray(src_emb, dtype=np.float32)
    edge_src = np.asarray(edge_src).astype(np.int64)
    edge_dst = np.asarray(edge_dst).astype(np.int64)
    n_dst = int(num_dst)
    n_src, d = src_emb.shape
    assert d == 64
    E = len(edge_dst)

    src_ext = np.concatenate(
        [src_emb.astype(BF16), np.zeros((1, 64), BF16)])  # zero row at n_src

    counts = np.bincount(edge_dst, minlength=n_dst)
    order = np.argsort(edge_dst, kind="stable")
    ss = edge_src[order]                      # edge srcs sorted by dst
    starts = np.zeros(n_dst + 1, dtype=np.int64)
    starts[1:] = np.cumsum(counts)

    sort_dst = np.argsort(-counts, kind="stable")
    sorted_counts = counts[sort_dst]

    nnz = int((counts > 0).sum())
    n_tiles = (nnz + 127) // 128              # tiles with at least one edge
    T_pad = (n_tiles + NCORES - 1) // NCORES  # ranks (tiles per core)

    # pad dst list so every (rank, core) has 128 rows; sentinel row = n_dst
    rows_all = np.full(T_pad * NCORES * 128, n_dst, dtype=np.int64)
    take = min(n_dst, n_tiles * 128)
    rows_all[:take] = sort_dst[:take]
    rows_all = rows_all.reshape(T_pad, NCORES, 128)

    counts_pad = np.concatenate([counts, [0]])
    starts_pad = np.concatenate([starts[:-1], [0]])

    L_ranks = tuple(
        int(max(sorted_counts[min(NCORES * j * 128, n_dst - 1)], 1))
        for j in range(T_pad))
    cols = 64 * int(sum(L_ranks))
    offs = np.concatenate(([0], np.cumsum([64 * L for L in L_ranks])))

    msgs = [np.zeros((128, cols), dtype=BF16) for _ in range(NCORES)]
    ar = np.arange(max(L_ranks))
    for j in range(T_pad):
        L = L_ranks[j]
        rows = rows_all[j].reshape(-1)                     # [8*128]
        st = starts_pad[rows]
        cnt = counts_pad[rows]
        eidx = st[:, None] + ar[None, :L]
        valid = ar[None, :L] < cnt[:, None]
        sidx = np.where(valid, ss[np.minimum(eidx, E - 1)], n_src)
        vals = src_ext[sidx]                               # [1024, L, 64]
        block = vals.reshape(NCORES, 128, 64 * L)          # slab-major
        o0, o1 = int(offs[j]), int(offs[j + 1])
        for c in range(NCORES):
            msgs[c][:, o0:o1] = block[c]

    if L_ranks not in _kernel_cache:
        _kernel_cache[L_ranks] = _build_kernel(L_ranks)
    nc = _kernel_cache[L_ranks]
    in_maps = [{"msg": msgs[c]} for c in range(NCORES)]
    res = run_bass_kernel_spmd(nc, in_maps, core_ids=list(range(NCORES)))

    full = np.zeros((n_dst + 1, 64), dtype=np.float32)
    for c in range(NCORES):
        blocks = np.asarray(res.results[c]["outp"]).astype(np.float32)
        blocks = blocks.reshape(128, T_pad, 64).transpose(1, 0, 2)
        full[rows_all[:, c, :].reshape(-1)] = blocks.reshape(-1, 64)
    return full[:n_dst]


if __name__ == "__main__":
    rng = np.random.default_rng(1)
    ns, nd, e = 1000, 1000, 5000
    semb = rng.standard_normal((ns, 64), dtype=np.float32)
    es = rng.integers(0, ns, e)
    ed = rng.integers(0, nd, e)
    got = kernel(src_emb=semb, edge_src=es, edge_dst=ed, num_dst=nd)
    exp = np.zeros((nd, 64), np.float32)
    np.add.at(exp, ed, semb[es])
    rel = np.abs(got - exp).max() / np.abs(exp).max()
    print("small-case rel err:", rel)


# revision 7
# speedup vs baseline: 2.4279x; 1.2414x over previous
"""GNN message passing (copy_u + segment_sum) on 8 Trainium2 cores.

Strategy (edge/data parallel, per the sharding hint):
  - Host: sort dst nodes by degree (desc); tiles of 128 dst rows each get a
    uniform slab depth L = max degree in tile.  Messages for tile t are packed
    slab-major [128 partitions = dst slot, L slabs x 64 feat] bf16 with zero
    padding for short segments.
  - Tiles are dealt round-robin to the 8 cores so every core runs the same
    program (rank j's depth = max L over that rank's 8 tiles).
  - Device: per tile, segment-sum = binary-tree halving with full-width
    unit-stride DVE tensor_tensor adds in place (eligible for 4x DVE mode);
    final level writes bf16 into the output tile.  No matmul, no one-hot;
    the kernel is DMA-bound streaming.
  - Host: scatter rows back (each dst lives in exactly one tile row).
"""
import sys
sys.path.insert(0, "/opt/trn_rl_repo")
import numpy as np
import ml_dtypes

import concourse.bass as bass
import concourse.bacc as bacc
import concourse.mybir as mybir
import concourse.tile as tile
from concourse.bass_utils import run_bass_kernel_spmd

NCORES = 8
GROUP = 14           # ranks per DMA group
BF16 = ml_dtypes.bfloat16

_kernel_cache = {}


def _build_kernel(L_groups):
    """L_groups: tuple of (n_tiles_in_group, L) — uniform slab depth per
    group so each tree level is one wide multi-tile DVE op."""
    bf16 = mybir.dt.bfloat16
    nc = bacc.Bacc("TRN2", target_bir_lowering=False, debug=False,
                   num_devices=NCORES)
    T = sum(gs for gs, _ in L_groups)
    cols = 64 * sum(gs * L for gs, L in L_groups)
    msg = nc.declare_dram_parameter("msg", [128, cols], bf16, isOutput=False)
    outp = nc.declare_dram_parameter("outp", [128, T * 64], bf16, isOutput=True)

    with tile.TileContext(nc) as tc:
        with tc.tile_pool(name="msgs", bufs=4) as mpool, \
             tc.tile_pool(name="ost", bufs=4) as opool:
            goff = 0
            t0 = 0
            for gs, L in L_groups:
                gcols = gs * 64 * L
                mt = mpool.tile([128, gcols], bf16, tag="mt")
                nc.sync.dma_start(out=mt[:], in_=msg[:, goff:goff + gcols])
                ot = opool.tile([128, gs * 64], bf16, tag="ot")
                m3 = mt[:].rearrange("p (t x) -> p t x", t=gs)
                o3 = ot[:].rearrange("p (t x) -> p t x", t=gs)
                n = L
                while n > 2:
                    h = n // 2
                    keep = n - h
                    nc.vector.tensor_tensor(
                        out=m3[:, :, :h * 64],
                        in0=m3[:, :, :h * 64],
                        in1=m3[:, :, keep * 64:n * 64],
                        op=mybir.AluOpType.add,
                    )
                    n = keep
                if n == 2:
                    nc.vector.tensor_tensor(
                        out=o3,
                        in0=m3[:, :, 0:64],
                        in1=m3[:, :, 64:128],
                        op=mybir.AluOpType.add,
                    )
                else:
                    nc.vector.tensor_copy(out=o3, in_=m3[:, :, 0:64])
                nc.scalar.dma_start(out=outp[:, t0 * 64:(t0 + gs) * 64],
                                    in_=ot[:])
                goff += gcols
                t0 += gs
    nc.compile()
    return nc


def kernel(src_emb, edge_src, edge_dst, num_dst):
    src_emb = np.asarray(src_emb, dtype=np.float32)
    edge_src = np.asarray(edge_src).astype(np.int64)
    edge_dst = np.asarray(edge_dst).astype(np.int64)
    n_dst = int(num_dst)
    n_src, d = src_emb.shape
    assert d == 64
    E = len(edge_dst)

    src_ext = np.concatenate(
        [src_emb.astype(BF16), np.zeros((1, 64), BF16)])  # zero row at n_src

    counts = np.bincount(edge_dst, minlength=n_dst)
    order = np.argsort(edge_dst, kind="stable")
    ss = edge_src[order]                      # edge srcs sorted by dst
    starts = np.zeros(n_dst + 1, dtype=np.int64)
    starts[1:] = np.cumsum(counts)

    sort_dst = np.argsort(-counts, kind="stable")
    sorted_counts = counts[sort_dst]

    nnz = int((counts > 0).sum())
    n_tiles = (nnz + 127) // 128              # tiles with at least one edge
    T_pad = (n_tiles + NCORES - 1) // NCORES  # ranks (tiles per core)

    # pad dst list so every (rank, core) has 128 rows; sentinel row = n_dst
    rows_all = np.full(T_pad * NCORES * 128, n_dst, dtype=np.int64)
    take = min(n_dst, n_tiles * 128)
    rows_all[:take] = sort_dst[:take]
    rows_all = rows_all.reshape(T_pad, NCORES, 128)

    counts_pad = np.concatenate([counts, [0]])
    starts_pad = np.concatenate([starts[:-1], [0]])

    # per-rank max degree, then uniform L per GROUP-sized chunk (pad up)
    L_rank = [int(max(sorted_counts[min(NCORES * j * 128, n_dst - 1)], 1))
              for j in range(T_pad)]
    L_groups = tuple(
        (min(GROUP, T_pad - g0), max(L_rank[g0:g0 + GROUP]))
        for g0 in range(0, T_pad, GROUP))
    L_ranks = tuple(L for gs, L in L_groups for _ in range(gs))
    cols = 64 * int(sum(L_ranks))
    offs = np.concatenate(([0], np.cumsum([64 * L for L in L_ranks])))

    msgs = [np.zeros((128, cols), dtype=BF16) for _ in range(NCORES)]
    ar = np.arange(max(L_ranks))
    for j in range(T_pad):
        L = L_ranks[j]
        rows = rows_all[j].reshape(-1)                     # [8*128]
        st = starts_pad[rows]
        cnt = counts_pad[rows]
        eidx = st[:, None] + ar[None, :L]
        valid = ar[None, :L] < cnt[:, None]
        sidx = np.where(valid, ss[np.minimum(eidx, E - 1)], n_src)
        vals = src_ext[sidx]                               # [1024, L, 64]
        block = vals.reshape(NCORES, 128, 64 * L)          # slab-major
        o0, o1 = int(offs[j]), int(offs[j + 1])
        for c in range(NCORES):
            msgs[c][:, o0:o1] = block[c]

    if L_groups not in _kernel_cache:
        _kernel_cache[L_groups] = _build_kernel(L_groups)
    nc = _kernel_cache[L_groups]
    in_maps = [{"msg": msgs[c]} for c in range(NCORES)]
    res = run_bass_kernel_spmd(nc, in_maps, core_ids=list(range(NCORES)))

    full = np.zeros((n_dst + 1, 64), dtype=np.float32)
    for c in range(NCORES):
        blocks = np.asarray(res.results[c]["outp"]).astype(np.float32)
        blocks = blocks.reshape(128, T_pad, 64).transpose(1, 0, 2)
        full[rows_all[:, c, :].reshape(-1)] = blocks.reshape(-1, 64)
    return full[:n_dst]


if __name__ == "__main__":
    rng = np.random.default_rng(1)
    ns, nd, e = 1000, 1000, 5000
    semb = rng.standard_normal((ns, 64), dtype=np.float32)
    es = rng.integers(0, ns, e)
    ed = rng.integers(0, nd, e)
    got = kernel(src_emb=semb, edge_src=es, edge_dst=ed, num_dst=nd)
    exp = np.zeros((nd, 64), np.float32)
    np.add.at(exp, ed, semb[es])
    rel = np.abs(got - exp).max() / np.abs(exp).max()
    print("small-case rel err:", rel)


# revision 8
# speedup vs baseline: 2.5452x; 1.0483x over previous
"""GNN message passing (copy_u + segment_sum) on 8 Trainium2 cores.

Strategy (edge/data parallel, per the sharding hint):
  - Host: sort dst nodes by degree (desc); tiles of 128 dst rows each get a
    uniform slab depth L = max degree in tile.  Messages for tile t are packed
    slab-major [128 partitions = dst slot, L slabs x 64 feat] bf16 with zero
    padding for short segments.
  - Tiles are dealt round-robin to the 8 cores so every core runs the same
    program (rank j's depth = max L over that rank's 8 tiles).
  - Device: per tile, segment-sum = binary-tree halving with full-width
    unit-stride DVE tensor_tensor adds in place (eligible for 4x DVE mode);
    final level writes bf16 into the output tile.  No matmul, no one-hot;
    the kernel is DMA-bound streaming.
  - Host: scatter rows back (each dst lives in exactly one tile row).
"""
import sys
sys.path.insert(0, "/opt/trn_rl_repo")
import numpy as np
import ml_dtypes

import concourse.bass as bass
import concourse.bacc as bacc
import concourse.mybir as mybir
import concourse.tile as tile
from concourse.bass_utils import run_bass_kernel_spmd

NCORES = 8
GROUP = 14           # ranks per DMA group
BF16 = ml_dtypes.bfloat16

_kernel_cache = {}


def _build_kernel(L_groups):
    """L_groups: tuple of (n_tiles_in_group, L) — uniform slab depth per
    group so each tree level is one wide multi-tile DVE op."""
    bf16 = mybir.dt.bfloat16
    nc = bacc.Bacc("TRN2", target_bir_lowering=False, debug=False,
                   num_devices=NCORES)
    T = sum(gs for gs, _ in L_groups)
    cols = 64 * sum(gs * L for gs, L in L_groups)
    msg = nc.declare_dram_parameter("msg", [128, cols], bf16, isOutput=False)
    outp = nc.declare_dram_parameter("outp", [128, T * 64], bf16, isOutput=True)

    with tile.TileContext(nc) as tc:
        with tc.tile_pool(name="msgs", bufs=4) as mpool, \
             tc.tile_pool(name="ost", bufs=4) as opool:
            goff = 0
            t0 = 0
            for gs, L in L_groups:
                gcols = gs * 64 * L
                mt = mpool.tile([128, gcols], bf16, tag="mt")
                nc.sync.dma_start(out=mt[:], in_=msg[:, goff:goff + gcols])
                ot = opool.tile([128, gs * 64], bf16, tag="ot")
                m3 = mt[:].rearrange("p (t x) -> p t x", t=gs)
                o3 = ot[:].rearrange("p (t x) -> p t x", t=gs)
                n = L
                while n > 2:
                    h = n // 2
                    keep = n - h
                    nc.vector.tensor_tensor(
                        out=m3[:, :, :h * 64],
                        in0=m3[:, :, :h * 64],
                        in1=m3[:, :, keep * 64:n * 64],
                        op=mybir.AluOpType.add,
                    )
                    n = keep
                if n == 2:
                    nc.vector.tensor_tensor(
                        out=o3,
                        in0=m3[:, :, 0:64],
                        in1=m3[:, :, 64:128],
                        op=mybir.AluOpType.add,
                    )
                else:
                    nc.vector.tensor_copy(out=o3, in_=m3[:, :, 0:64])
                nc.scalar.dma_start(out=outp[:, t0 * 64:(t0 + gs) * 64],
                                    in_=ot[:])
                goff += gcols
                t0 += gs
    nc.compile()
    return nc


def kernel(src_emb, edge_src, edge_dst, num_dst):
    src_emb = np.asarray(src_emb, dtype=np.float32)
    edge_src = np.asarray(edge_src).astype(np.int64)
    edge_dst = np.asarray(edge_dst).astype(np.int64)
    n_dst = int(num_dst)
    n_src, d = src_emb.shape
    assert d == 64
    E = len(edge_dst)

    src_ext = np.concatenate(
        [src_emb.astype(BF16), np.zeros((1, 64), BF16)])  # zero row at n_src

    counts = np.bincount(edge_dst, minlength=n_dst)
    order = np.argsort(edge_dst, kind="stable")
    ss = edge_src[order]                      # edge srcs sorted by dst
    starts = np.zeros(n_dst + 1, dtype=np.int64)
    starts[1:] = np.cumsum(counts)

    sort_dst = np.argsort(-counts, kind="stable")
    sorted_counts = counts[sort_dst]

    nnz = int((counts > 0).sum())
    n_tiles = (nnz + 127) // 128              # tiles with at least one edge
    T_pad = (n_tiles + NCORES - 1) // NCORES  # ranks (tiles per core)

    # pad dst list so every (rank, core) has 128 rows; sentinel row = n_dst
    rows_all = np.full(T_pad * NCORES * 128, n_dst, dtype=np.int64)
    take = min(n_dst, n_tiles * 128)
    rows_all[:take] = sort_dst[:take]
    rows_all = rows_all.reshape(T_pad, NCORES, 128)

    counts_pad = np.concatenate([counts, [0]])
    starts_pad = np.concatenate([starts[:-1], [0]])

    # per-rank max degree (ranks sorted desc by construction)
    L_rank = [int(max(sorted_counts[min(NCORES * j * 128, n_dst - 1)], 1))
              for j in range(T_pad)]

    # greedy groups: uniform L per group (pad up), <=4% padding, <=1MB, <=16
    CAP_BYTES = 1_000_000
    bounds = []
    i = 0
    while i < T_pad:
        L = L_rank[i]
        j = i + 1
        while j < T_pad and j - i < 16:
            gs = j + 1 - i
            pad = gs * L - sum(L_rank[i:j + 1])
            if pad > 0.04 * gs * L or gs * L * 16384 > CAP_BYTES:
                break
            j += 1
        bounds.append((i, j, L))
        i = j
    # pyramid emit order: small -> large -> small
    by_size = sorted(range(len(bounds)),
                     key=lambda k: (bounds[k][1] - bounds[k][0]) * bounds[k][2])
    emit = by_size[0::2] + by_size[1::2][::-1]

    L_groups = tuple((bounds[k][1] - bounds[k][0], bounds[k][2]) for k in emit)
    perm = np.concatenate([np.arange(bounds[k][0], bounds[k][1])
                           for k in emit])
    rows_all = rows_all[perm]
    L_ranks = tuple(L for gs, L in L_groups for _ in range(gs))
    cols = 64 * int(sum(L_ranks))
    offs = np.concatenate(([0], np.cumsum([64 * L for L in L_ranks])))

    msgs = [np.zeros((128, cols), dtype=BF16) for _ in range(NCORES)]
    ar = np.arange(max(L_ranks))
    for j in range(T_pad):
        L = L_ranks[j]
        rows = rows_all[j].reshape(-1)                     # [8*128]
        st = starts_pad[rows]
        cnt = counts_pad[rows]
        eidx = st[:, None] + ar[None, :L]
        valid = ar[None, :L] < cnt[:, None]
        sidx = np.where(valid, ss[np.minimum(eidx, E - 1)], n_src)
        vals = src_ext[sidx]                               # [1024, L, 64]
        block = vals.reshape(NCORES, 128, 64 * L)          # slab-major
        o0, o1 = int(offs[j]), int(offs[j + 1])
        for c in range(NCORES):
            msgs[c][:, o0:o1] = block[c]

    if L_groups not in _kernel_cache:
        _kernel_cache[L_groups] = _build_kernel(L_groups)
    nc = _kernel_cache[L_groups]
    in_maps = [{"msg": msgs[c]} for c in range(NCORES)]
    res = run_bass_kernel_spmd(nc, in_maps, core_ids=list(range(NCORES)))

    full = np.zeros((n_dst + 1, 64), dtype=np.float32)
    for c in range(NCORES):
        blocks = np.asarray(res.results[c]["outp"]).astype(np.float32)
        blocks = blocks.reshape(128, T_pad, 64).transpose(1, 0, 2)
        full[rows_all[:, c, :].reshape(-1)] = blocks.reshape(-1, 64)
    return full[:n_dst]


if __name__ == "__main__":
    rng = np.random.default_rng(1)
    ns, nd, e = 1000, 1000, 5000
    semb = rng.standard_normal((ns, 64), dtype=np.float32)
    es = rng.integers(0, ns, e)
    ed = rng.integers(0, nd, e)
    got = kernel(src_emb=semb, edge_src=es, edge_dst=ed, num_dst=nd)
    exp = np.zeros((nd, 64), np.float32)
    np.add.at(exp, ed, semb[es])
    rel = np.abs(got - exp).max() / np.abs(exp).max()
    print("small-case rel err:", rel)
